# revision 12
# baseline (speedup 1.0000x reference)
"""Trainium2 Bass kernel for the bipartite GCNN (8 NeuronCores, SPMD).

v2 design. Math identical to the reference:
  colE = relu(col_features @ col_W + col_b); rowE likewise.
  v2c: t1 = colA[ci] + rowC[ri] + ef*w1b + b1, colA = colE@W1a_v,
       rowC = rowE@W1c_v (premultiplied tables).
       msg = relu(t1); new_row = rowE + segsum(msg, ri) @ W2 + deg*b2
  c2v symmetric with rowA = new_rowE@W1a_c, colCp = colE@W1c_c.
  scores = new_col @ out_W + out_b

Sharding: destination-range; core k owns dest nodes [6250k, 6250(k+1)).
Edges sorted by (core, lo/hi-of-source, 64-wide dest window, source);
runs padded to a shared chunk plan so the SPMD program is identical.

Per 128-edge chunk (b1 baked into colA/colCp at phase0):
  DVE:  t1 = ef*w1b + gA + gL (3 TTs), S = (dloc == iota64) one-hot (64 wide)
  Act:  msg = relu(t1) -> bf16
  PE :  psum_seg[64, w%8*64:+64] += msg.T @ S   (the only per-chunk matmul)
  Act:  psum_seg -> seg (bf16), one copy per 8 windows
"""

import numpy as np
import ml_dtypes

import concourse.bass as bass
import concourse.mybir as mybir
import concourse.tile as tile
from concourse import bacc
from concourse.bass_utils import run_bass_kernel_spmd

NC = 8
N = 50000
SLICE = 6250
WW = 64                    # dest window width
NW = 98                    # windows per slice (98*64 = 6272)
SLICEP = NW * WW           # 6272
TROWS = NC * SLICEP        # 50176
OWNP = SLICEP              # own-block width for phase0 stripes
H = 64
LOHI = 32768
GC = 8                     # gather-call granularity in chunks (1024 descs)
ST = 8                     # compute supertile in chunks
IDXC = 8                   # gather calls per idx-load batch

FP32 = mybir.dt.float32
BF16 = mybir.dt.bfloat16
I16 = mybir.dt.int16
BF = ml_dtypes.bfloat16


# ----------------------------------------------------------------------------
# host-side preprocessing
# ----------------------------------------------------------------------------

def _g_of(n):
    return SLICEP * (n // SLICE) + n % SLICE


def _idx_layout(a):
    """slot array [E_PAD] -> dma_gather idx layout [128, E_PAD//16] int16"""
    A = a.reshape(-1, 16).T  # [16, E/16]
    return np.tile(A, (8, 1)).copy()


def _build_direction(dest, gidx, ef):
    core = dest // SLICE
    dl = dest - SLICE * core
    w = dl >> 6
    dloc = dl & 63
    sec = (gidx >= LOHI).astype(np.int64)

    key = (core * 2 + sec) * NW + w
    order = np.lexsort((gidx, key))

    cnt = np.bincount(key[order], minlength=NC * 2 * NW).reshape(NC, 2, NW)
    wch = np.maximum(1, -(-cnt.max(axis=0) // 128))  # [2, NW]
    chunks_lo = wch[0]
    chunks_hi = wch[1]
    n_chunks = int(chunks_lo.sum() + chunks_hi.sum())
    E_PAD = 128 * n_chunks

    group_chunks = np.concatenate([chunks_lo, chunks_hi])
    group_off = np.zeros(2 * NW, dtype=np.int64)
    group_off[1:] = np.cumsum(group_chunks)[:-1] * 128

    per_core = []
    for k in range(NC):
        sel = order[core[order] == k]
        kgrp = sec[sel] * NW + w[sel]
        kcnt = np.bincount(kgrp, minlength=2 * NW)
        within = (
            np.concatenate([np.arange(c) for c in kcnt])
            if len(sel)
            else np.zeros(0, np.int64)
        )
        slot = group_off[kgrp] + within

        a_ef = np.zeros(E_PAD, dtype=np.float32)
        a_dloc = np.full(E_PAD, 200, dtype=np.float32)
        a_g16 = np.zeros(E_PAD, dtype=np.int16)
        a_l16 = np.zeros(E_PAD, dtype=np.int16)

        a_ef[slot] = ef[sel]
        a_dloc[slot] = dloc[sel]
        a_g16[slot] = (gidx[sel] - sec[sel] * LOHI).astype(np.int16)
        a_l16[slot] = dl[sel].astype(np.int16)

        per_core.append(
            dict(
                g16=_idx_layout(a_g16),
                l16=_idx_layout(a_l16),
                ef=a_ef.reshape(-1, 128).T.astype(BF).copy(),      # [128, nch]
                dloc=a_dloc.reshape(-1, 128).T.astype(BF).copy(),  # [128, nch]
            )
        )

    deg = np.bincount(dest, minlength=N).astype(np.float32)
    deg_local = np.zeros((NC, 1, SLICEP), np.float32)
    for k in range(NC):
        deg_local[k, 0, :SLICE] = deg[k * SLICE : (k + 1) * SLICE]

    meta = dict(
        chunks_lo=[int(x) for x in chunks_lo],
        chunks_hi=[int(x) for x in chunks_hi],
        n_chunks=n_chunks,
    )
    return meta, per_core, deg_local.astype(BF)


def _pad_features_blocks(feat):
    D = feat.shape[1]
    out = np.zeros((D, TROWS), np.float32)
    for k in range(NC):
        out[:, k * SLICEP : k * SLICEP + SLICE] = feat[k * SLICE : (k + 1) * SLICE].T
    return out.astype(BF)


def host_prep(inputs):
    ri = np.asarray(inputs["edge_indices"][0]).astype(np.int64)
    ci = np.asarray(inputs["edge_indices"][1]).astype(np.int64)
    ef = np.asarray(inputs["edge_features"]).reshape(-1).astype(np.float32)

    meta_v, pc_v, deg_r = _build_direction(ri, _g_of(ci), ef)
    meta_c, pc_c, deg_c = _build_direction(ci, _g_of(ri), ef)

    colF = np.asarray(inputs["col_features"], np.float32)
    rowF = np.asarray(inputs["row_features"], np.float32)

    colFT_own = np.zeros((NC, 19, OWNP), np.float32)
    rowFT_own = np.zeros((NC, 14, OWNP), np.float32)
    for k in range(NC):
        colFT_own[k, :, :SLICE] = colF[k * SLICE : (k + 1) * SLICE].T
        rowFT_own[k, :, :SLICE] = rowF[k * SLICE : (k + 1) * SLICE].T

    return dict(
        meta_v=meta_v, pc_v=pc_v, deg_r=deg_r,
        meta_c=meta_c, pc_c=pc_c, deg_c=deg_c,
        colFT=_pad_features_blocks(colF),
        colFT_own=colFT_own.astype(BF), rowFT_own=rowFT_own.astype(BF),
    )


def host_weights(inputs):
    f = lambda x: np.asarray(x, np.float32)
    b = lambda x: np.asarray(x, np.float32).astype(BF)
    v2c_W1 = f(inputs["v2c_W1"]); c2v_W1 = f(inputs["c2v_W1"])
    w = dict(
        col_W=b(inputs["col_W"]),
        col_b=f(inputs["col_b"]).reshape(64, 1),
        row_W=b(inputs["row_W"]),
        row_b=f(inputs["row_b"]).reshape(64, 1),
        W1a_v=v2c_W1[:64].astype(BF).copy(),
        W1c_v=v2c_W1[65:129].astype(BF).copy(),
        w1b_v=np.tile(v2c_W1[64:65], (128, 1)).astype(BF),
        b1row_v=f(inputs["v2c_b1"]).reshape(1, 64).astype(BF),
        W2aug_v=np.vstack([f(inputs["v2c_W2"]), f(inputs["v2c_b2"])[None, :]]).astype(BF),
        W2_v=b(inputs["v2c_W2"]),
        W1a_c=c2v_W1[:64].astype(BF).copy(),
        W1c_c=c2v_W1[65:129].astype(BF).copy(),
        w1b_c=np.tile(c2v_W1[64:65], (128, 1)).astype(BF),
        b1row_c=f(inputs["c2v_b1"]).reshape(1, 64).astype(BF),
        W2aug_c=np.vstack([f(inputs["c2v_W2"]), f(inputs["c2v_b2"])[None, :]]).astype(BF),
        W2_c=b(inputs["c2v_W2"]),
        out_W=b(inputs["out_W"]),
        out_b=f(inputs["out_b"]).reshape(1, 1),
        ones128=np.ones((1, 128), np.float32).astype(BF),
        iota64=np.tile(np.arange(64, dtype=np.float32)[None, :], (128, 1)).astype(BF),
    )
    return w


# ----------------------------------------------------------------------------
# kernel builder
# ----------------------------------------------------------------------------

def _chunk_plan(meta):
    """list over chunks of (sec, window, first_of_window, last_of_window)"""
    cw = []
    for sec, chunks in enumerate((meta["chunks_lo"], meta["chunks_hi"])):
        for w, c in enumerate(chunks):
            for j in range(c):
                cw.append((sec, w, j == 0, j == c - 1))
    return cw


def _call_plan(meta):
    """gather calls: (chunk0, nchunks) not crossing the lo/hi boundary"""
    nlo = sum(meta["chunks_lo"])
    nhi = sum(meta["chunks_hi"])
    calls = []
    for base, n in ((0, nlo), (nlo, nhi)):
        c = 0
        while c < n:
            cc = min(GC, n - c)
            calls.append((base + c, cc))
            c += cc
    return calls


STRIPES = [(i * 512, 512) for i in range(12)] + [(6144, 128)]


def build_kernel(meta_v, meta_c, repeat=1, skip_gathers=False):
    """repeat>1 builds a TIMING variant: both edge phases + epilogues wrapped
    in a hardware loop (collective hoisted out; scores not meaningful)."""
    nc = bacc.Bacc("TRN2", target_bir_lowering=False, debug=False, num_devices=NC,
                   dynamic_dma_scratch_size=32768, num_swdge_queues=2)

    def din(name, shape, dt=FP32):
        return nc.dram_tensor(name, shape, dt, kind="ExternalInput")

    colFT = din("colFT", [19, TROWS], BF16)
    colFT_own = din("colFT_own", [19, OWNP], BF16)
    rowFT_own = din("rowFT_own", [14, OWNP], BF16)
    col_W = din("col_W", [19, 64], BF16); col_b = din("col_b", [64, 1])
    row_W = din("row_W", [14, 64], BF16); row_b = din("row_b", [64, 1])
    W1a_v = din("W1a_v", [64, 64], BF16); W1c_v = din("W1c_v", [64, 64], BF16)
    w1b_v = din("w1b_v", [128, 64], BF16); b1row_v = din("b1row_v", [1, 64], BF16)
    W2aug_v = din("W2aug_v", [65, 64], BF16); W2_v = din("W2_v", [64, 64], BF16)
    W1a_c = din("W1a_c", [64, 64], BF16); W1c_c = din("W1c_c", [64, 64], BF16)
    w1b_c = din("w1b_c", [128, 64], BF16); b1row_c = din("b1row_c", [1, 64], BF16)
    W2aug_c = din("W2aug_c", [65, 64], BF16); W2_c = din("W2_c", [64, 64], BF16)
    out_W = din("out_W", [64, 1], BF16); out_b = din("out_b", [1, 1])
    ones128 = din("ones128", [1, 128], BF16)
    iota64 = din("iota64", [128, 64], BF16)

    ncv = meta_v["n_chunks"]; ncc = meta_c["n_chunks"]
    g16_v = din("g16_v", [128, ncv * 8], I16)
    l16_v = din("l16_v", [128, ncv * 8], I16)
    ef_v = din("ef_v", [128, ncv], BF16)
    dloc_v = din("dloc_v", [128, ncv], BF16)
    deg_r = din("deg_r", [1, SLICEP], BF16)
    g16_c = din("g16_c", [128, ncc * 8], I16)
    l16_c = din("l16_c", [128, ncc * 8], I16)
    ef_c = din("ef_c", [128, ncc], BF16)
    dloc_c = din("dloc_c", [128, ncc], BF16)
    deg_c = din("deg_c", [1, SLICEP], BF16)

    scores = nc.dram_tensor("scores", [SLICEP], FP32, kind="ExternalOutput")

    RELU = mybir.ActivationFunctionType.Relu
    COPY = mybir.ActivationFunctionType.Copy
    ADD = mybir.AluOpType.add
    MULT = mybir.AluOpType.mult
    EQ = mybir.AluOpType.is_equal

    with tile.TileContext(nc) as tc:
        with (
            tc.tile_pool(name="consts", bufs=1) as consts,
            tc.tile_pool(name="sb", bufs=2) as sb,
            tc.tile_pool(name="gath", bufs=2) as gath,
            tc.tile_pool(name="seg", bufs=1) as segp,
            tc.tile_pool(name="et", bufs=1) as etp,
            tc.tile_pool(name="ps_a", bufs=2, space="PSUM") as ps_a,
            tc.tile_pool(name="ps_b", bufs=2, space="PSUM") as ps_b,
            tc.tile_pool(name="ps_c", bufs=2, space="PSUM") as ps_c,
            tc.tile_pool(name="dram", bufs=1, space="DRAM") as dram,
        ):
            # ---- DRAM scratch (gathered tables are fp32: 256B rows)
            colA_t = dram.tile([TROWS, 64], FP32)
            rowC_t = dram.tile([SLICEP, 64], FP32)
            colCp_t = dram.tile([SLICEP, 64], FP32)
            rowA_slice = dram.tile([SLICEP, 64], FP32)
            rowA_full = dram.tile([TROWS, 64], FP32)

            # ---- small consts
            def cload(dram_h, shape, dt=FP32):
                t = consts.tile(shape, dt, tag=f"c_{dram_h.name}")
                nc.sync.dma_start(t[:], dram_h[:])
                return t

            colW_s = cload(col_W, [19, 64], BF16); colb_s = cload(col_b, [64, 1])
            rowW_s = cload(row_W, [14, 64], BF16); rowb_s = cload(row_b, [64, 1])
            W1av_s = cload(W1a_v, [64, 64], BF16); W1cv_s = cload(W1c_v, [64, 64], BF16)
            w1bv_s = cload(w1b_v, [128, 64], BF16); b1v_s = cload(b1row_v, [1, 64], BF16)
            W2augv_s = cload(W2aug_v, [65, 64], BF16); W2v_s = cload(W2_v, [64, 64], BF16)
            W1ac_s = cload(W1a_c, [64, 64], BF16); W1cc_s = cload(W1c_c, [64, 64], BF16)
            w1bc_s = cload(w1b_c, [128, 64], BF16); b1c_s = cload(b1row_c, [1, 64], BF16)
            W2augc_s = cload(W2aug_c, [65, 64], BF16); W2c_s = cload(W2_c, [64, 64], BF16)
            outW_s = cload(out_W, [64, 1], BF16); outb_s = cload(out_b, [1, 1])
            ones_s = cload(ones128, [1, 128], BF16)
            iota_s = cload(iota64, [128, 64], BF16)

            # embedding tables for the epilogues, kept in SBUF
            colET = etp.tile([64, OWNP], BF16, tag="colET")
            rowET = etp.tile([64, OWNP], BF16, tag="rowET")

            # ---- phase 0: premultiplied node tables
            def emit_table(featT, D, Wemb_s, bemb_s, stripes, Wtab_s, table,
                           embT_out, brow_s=None):
                """embT = relu(Wemb.T @ featT + bemb); table = embT.T @ Wtab
                (+ brow broadcast over rows, via a rank-1 ones matmul)."""
                for (o, L) in stripes:
                    ft = sb.tile([D, 512], BF16, tag="ph0_ft")
                    nc.sync.dma_start(ft[:, :L], featT[:, o : o + L])
                    pe = ps_a.tile([64, 512], FP32, tag="ps_a")
                    nc.tensor.matmul(pe[:, :L], lhsT=Wemb_s[:], rhs=ft[:, :L],
                                     start=True, stop=True)
                    if embT_out is not None:
                        embT = embT_out[:, o : o + L]
                        nc.scalar.activation(embT, pe[:, :L], RELU, bias=bemb_s[:, :1])
                    else:
                        et = sb.tile([64, 512], BF16, tag="ph0_emb")
                        embT = et[:, :L]
                        nc.scalar.activation(embT, pe[:, :L], RELU, bias=bemb_s[:, :1])
                    nch = L // 128
                    pa = ps_b.tile([128, 8, 64], FP32, tag="ps_b")
                    for c in range(nch):
                        nc.tensor.matmul(
                            pa[:, c, :],
                            lhsT=embT[:, 128 * c : 128 * (c + 1)],
                            rhs=Wtab_s[:], start=True, stop=(brow_s is None),
                        )
                        if brow_s is not None:
                            nc.tensor.matmul(
                                pa[:, c, :], lhsT=ones_s[:], rhs=brow_s[:],
                                start=False, stop=True,
                            )
                    stage = sb.tile([128, 4, 64], FP32, tag="ph0_stage")
                    nc.scalar.activation(stage[:, :nch, :], pa[:, :nch, :], COPY)
                    nc.sync.dma_start(
                        table[o : o + L, :].rearrange("(c p) h -> p c h", p=128),
                        stage[:, :nch, :],
                    )

            colA_stripes = [(i * 512, 512) for i in range(TROWS // 512)]    # 98
            emit_table(colFT, 19, colW_s, colb_s, colA_stripes, W1av_s, colA_t[:],
                       None, b1v_s)
            emit_table(colFT_own, 19, colW_s, colb_s, STRIPES, W1cc_s,
                       colCp_t[:], colET, b1c_s)
            emit_table(rowFT_own, 14, rowW_s, rowb_s, STRIPES, W1cv_s,
                       rowC_t[:], rowET)

            # ---- edge phase
            def edge_phase(meta, tab_lo, tab_hi, tab_loc, g16_d, l16_d, ef_d,
                           dloc_d, w1b_s, deg_d):
                cw = _chunk_plan(meta)
                calls = _call_plan(meta)
                nlo = sum(meta["chunks_lo"])
                nch = meta["n_chunks"]

                seg_lo = segp.tile([65, SLICEP], BF16, tag="seg_lo")
                seg_hi = segp.tile([64, SLICEP], BF16, tag="seg_hi")
                nc.sync.dma_start(seg_lo[64:65, :], deg_d[:])

                dlt_s = segp.tile([128, nch], BF16, tag="dlt")
                nc.sync.dma_start(dlt_s[:], dloc_d[:])
                eft_s = segp.tile([128, nch], BF16, tag="eft")
                nc.sync.dma_start(eft_s[:], ef_d[:])

                pw = None
                gt = lt = None
                idx_base = -1
                for ci_, (c0, ncall) in enumerate(calls):
                    is_lo = c0 < nlo
                    # batched idx loads (IDXC calls worth)
                    if ci_ % IDXC == 0:
                        nchb = 0
                        for (cc0, cnn) in calls[ci_ : ci_ + IDXC]:
                            nchb += cnn
                        nidxb = 128 * nchb
                        idx_base = c0
                        gt = sb.tile([128, GC * IDXC * 8], I16, tag="gidx")
                        nc.sync.dma_start(
                            gt[:, : nidxb // 16],
                            g16_d[:, c0 * 8 : c0 * 8 + nidxb // 16])
                        lt = sb.tile([128, GC * IDXC * 8], I16, tag="lidx")
                        nc.sync.dma_start(
                            lt[:, : nidxb // 16],
                            l16_d[:, c0 * 8 : c0 * 8 + nidxb // 16])

                    nidx = 128 * ncall
                    ib = (c0 - idx_base) * 8
                    gA = gath.tile([128, GC, 64], FP32, tag="gA")
                    gL = gath.tile([128, GC, 64], FP32, tag="gL")
                    if not skip_gathers:
                        nc.gpsimd.dma_gather(
                            gA[:, :ncall, :], (tab_lo if is_lo else tab_hi),
                            gt[:, ib : ib + nidx // 16],
                            num_idxs=nidx, num_idxs_reg=nidx, elem_size=64,
                        )
                        nc.gpsimd.dma_gather(
                            gL[:, :ncall, :], tab_loc,
                            lt[:, ib : ib + nidx // 16],
                            num_idxs=nidx, num_idxs_reg=nidx, elem_size=64,
                        )

                    for t0 in range(0, ncall, ST):
                        g = min(ST, ncall - t0)
                        cbase = c0 + t0

                        t1 = sb.tile([128, ST, 64], BF16, tag="t1")
                        nc.vector.tensor_tensor(
                            t1[:, :g, :],
                            w1b_s[:, None, :].to_broadcast([128, g, 64]),
                            eft_s[:, cbase : cbase + g, None].to_broadcast(
                                [128, g, 64]),
                            op=MULT,
                        )
                        nc.vector.tensor_tensor(
                            t1[:, :g, :], t1[:, :g, :], gA[:, t0 : t0 + g, :],
                            op=ADD,
                        )
                        nc.vector.tensor_tensor(
                            t1[:, :g, :], t1[:, :g, :], gL[:, t0 : t0 + g, :],
                            op=ADD,
                        )
                        S = sb.tile([128, ST, 64], BF16, tag="S")
                        nc.vector.tensor_tensor(
                            S[:, :g, :],
                            dlt_s[:, cbase : cbase + g, None].to_broadcast(
                                [128, g, 64]),
                            iota_s[:, None, :].to_broadcast([128, g, 64]),
                            op=EQ,
                        )

                        msg = sb.tile([128, ST, 64], BF16, tag="msg")
                        nc.scalar.activation(msg[:, :g, :], t1[:, :g, :], RELU)

                        for j in range(g):
                            c = cbase + j
                            sec, w, first, last = cw[c]
                            if w % 8 == 0 and first:
                                pw = ps_c.tile([64, 8, 64], FP32, tag="ps_c")
                            nc.tensor.matmul(
                                pw[:, w % 8, :], lhsT=msg[:, j, :], rhs=S[:, j, :],
                                start=first, stop=last,
                            )
                            if last and (w % 8 == 7 or w == NW - 1):
                                gw = 8 if w % 8 == 7 else (w % 8) + 1
                                wg = w - (w % 8)
                                acc = seg_lo if sec == 0 else seg_hi
                                nc.scalar.activation(
                                    acc[0:64, WW * wg : WW * (wg + gw)],
                                    pw[:, :gw, :], COPY,
                                )
                return seg_lo, seg_hi

            def epilogue(seg_lo, seg_hi, W2aug_s, W2_s_, ET, is_v2c):
                for (o, L) in STRIPES:
                    pn = ps_a.tile([64, 512], FP32, tag="ps_a")
                    nc.tensor.matmul(pn[:, :L], lhsT=W2aug_s[:],
                                     rhs=seg_lo[:, o : o + L], start=True, stop=False)
                    nc.tensor.matmul(pn[:, :L], lhsT=W2_s_[:],
                                     rhs=seg_hi[:, o : o + L], start=False, stop=True)
                    nT = sb.tile([64, 512], BF16, tag="nT")
                    nc.vector.tensor_tensor(nT[:, :L], pn[:, :L], ET[:, o : o + L],
                                            op=ADD)
                    if is_v2c:
                        nch = L // 128
                        pa = ps_b.tile([128, 8, 64], FP32, tag="ps_b")
                        for c in range(nch):
                            nc.tensor.matmul(
                                pa[:, c, :],
                                lhsT=nT[:, 128 * c : 128 * (c + 1)],
                                rhs=W1ac_s[:], start=True, stop=True,
                            )
                        stage = sb.tile([128, 4, 64], FP32, tag="rA_stage")
                        nc.scalar.activation(stage[:, :nch, :], pa[:, :nch, :], COPY)
                        nc.sync.dma_start(
                            rowA_slice[o : o + L, :].rearrange("(c p) h -> p c h",
                                                               p=128),
                            stage[:, :nch, :],
                        )
                    else:
                        psc = ps_c.tile([1, 512], FP32, tag="ps_c")
                        nc.tensor.matmul(psc[:, :L], lhsT=outW_s[:], rhs=nT[:, :L],
                                         start=True, stop=True)
                        sct = sb.tile([1, 512], FP32, tag="sc")
                        nc.vector.tensor_scalar(
                            out=sct[:, :L], in0=psc[:, :L],
                            scalar1=outb_s[:1, :1], scalar2=None, op0=ADD,
                        )
                        nc.sync.dma_start(scores[o : o + L], sct[:, :L])

            def v2c_block():
                seg_lo, seg_hi = edge_phase(
                    meta_v, colA_t[0:LOHI, :], colA_t[LOHI:TROWS, :], rowC_t[:],
                    g16_v, l16_v, ef_v, dloc_v, w1bv_s, deg_r,
                )
                epilogue(seg_lo, seg_hi, W2augv_s, W2v_s, rowET, True)

            def c2v_block():
                seg_lo, seg_hi = edge_phase(
                    meta_c, rowA_full[0:LOHI, :], rowA_full[LOHI:TROWS, :],
                    colCp_t[:], g16_c, l16_c, ef_c, dloc_c, w1bc_s, deg_c,
                )
                epilogue(seg_lo, seg_hi, W2augc_s, W2c_s, colET, False)

            def do_collective():
                nc.gpsimd.collective_compute(
                    "AllGather",
                    mybir.AluOpType.bypass,
                    replica_groups=[list(range(NC))],
                    ins=[rowA_slice.opt()],
                    outs=[rowA_full.opt()],
                )

            if repeat == 1:
                v2c_block()
                do_collective()
                c2v_block()
            else:
                v2c_block()
                do_collective()
                with tc.For_i(0, repeat, 1):
                    c2v_block()
                    v2c_block()

    nc.compile()
    return nc


# ----------------------------------------------------------------------------
# entry point
# ----------------------------------------------------------------------------

_CACHE = {}


def _get_kernel(meta_v, meta_c):
    key = (
        tuple(meta_v["chunks_lo"]), tuple(meta_v["chunks_hi"]),
        tuple(meta_c["chunks_lo"]), tuple(meta_c["chunks_hi"]),
    )
    if key not in _CACHE:
        _CACHE[key] = build_kernel(meta_v, meta_c)
    return _CACHE[key]


def make_in_maps(inputs, prep):
    w = host_weights(inputs)
    shared = dict(
        colFT=prep["colFT"],
        col_W=w["col_W"], col_b=w["col_b"], row_W=w["row_W"], row_b=w["row_b"],
        W1a_v=w["W1a_v"], W1c_v=w["W1c_v"], w1b_v=w["w1b_v"],
        b1row_v=w["b1row_v"],
        W2aug_v=w["W2aug_v"], W2_v=w["W2_v"],
        W1a_c=w["W1a_c"], W1c_c=w["W1c_c"], w1b_c=w["w1b_c"],
        b1row_c=w["b1row_c"],
        W2aug_c=w["W2aug_c"], W2_c=w["W2_c"],
        out_W=w["out_W"], out_b=w["out_b"], ones128=w["ones128"],
        iota64=w["iota64"],
    )
    in_maps = []
    for k in range(NC):
        pv, pc = prep["pc_v"][k], prep["pc_c"][k]
        m = dict(
            shared,
            colFT_own=prep["colFT_own"][k],
            rowFT_own=prep["rowFT_own"][k],
            g16_v=pv["g16"], l16_v=pv["l16"], ef_v=pv["ef"], dloc_v=pv["dloc"],
            deg_r=prep["deg_r"][k],
            g16_c=pc["g16"], l16_c=pc["l16"], ef_c=pc["ef"], dloc_c=pc["dloc"],
            deg_c=prep["deg_c"][k],
        )
        in_maps.append({kk: np.ascontiguousarray(vv) for kk, vv in m.items()})
    return in_maps


def kernel(**inputs):
    prep = host_prep(inputs)
    nc = _get_kernel(prep["meta_v"], prep["meta_c"])
    in_maps = make_in_maps(inputs, prep)
    res = run_bass_kernel_spmd(nc, in_maps, core_ids=list(range(NC)))
    out = np.zeros(N, np.float32)
    for k in range(NC):
        out[k * SLICE : (k + 1) * SLICE] = np.asarray(
            res.results[k]["scores"]).reshape(-1)[:SLICE]
    return out


# revision 18
# speedup vs baseline: 2.3879x; 2.3879x over previous
"""Trainium2 Bass kernel for the bipartite GCNN (8 NeuronCores, SPMD).

v2 design. Math identical to the reference:
  colE = relu(col_features @ col_W + col_b); rowE likewise.
  v2c: t1 = colA[ci] + rowC[ri] + ef*w1b + b1, colA = colE@W1a_v,
       rowC = rowE@W1c_v (premultiplied tables).
       msg = relu(t1); new_row = rowE + segsum(msg, ri) @ W2 + deg*b2
  c2v symmetric with rowA = new_rowE@W1a_c, colCp = colE@W1c_c.
  scores = new_col @ out_W + out_b

Sharding: destination-range; core k owns dest nodes [6250k, 6250(k+1)).
Edges sorted by (core, lo/hi-of-source, 64-wide dest window, source);
runs padded to a shared chunk plan so the SPMD program is identical.

Per 128-edge chunk (b1 baked into colA/colCp at phase0):
  DVE:  t1 = ef*w1b + gA + gL (3 TTs), S = (dloc == iota64) one-hot (64 wide)
  Act:  msg = relu(t1) -> bf16
  PE :  psum_seg[64, w%8*64:+64] += msg.T @ S   (the only per-chunk matmul)
  Act:  psum_seg -> seg (bf16), one copy per 8 windows
"""

import numpy as np
import ml_dtypes

import concourse.bass as bass
import concourse.mybir as mybir
import concourse.tile as tile
from concourse import bacc
from concourse.bass_utils import run_bass_kernel_spmd

NC = 8
N = 50000
SLICE = 6250
WW = 64                    # dest window width
NW = 98                    # windows per slice (98*64 = 6272)
SLICEP = NW * WW           # 6272
TROWS = NC * SLICEP        # 50176
OWNP = SLICEP              # own-block width for phase0 stripes
H = 64
LOHI = 32768
GC = 8                     # gather-call granularity in chunks (1024 descs)
ST = 8                     # compute supertile in chunks
IDXC = 8                   # gather calls per idx-load batch

FP32 = mybir.dt.float32
BF16 = mybir.dt.bfloat16
I16 = mybir.dt.int16
BF = ml_dtypes.bfloat16


# ----------------------------------------------------------------------------
# host-side preprocessing
# ----------------------------------------------------------------------------

def _g_of(n):
    return SLICEP * (n // SLICE) + n % SLICE


def _idx_layout(a):
    """slot array [E_PAD] -> dma_gather idx layout [128, E_PAD//16] int16"""
    A = a.reshape(-1, 16).T  # [16, E/16]
    return np.tile(A, (8, 1)).copy()


def _build_direction(dest, gidx, ef):
    core = dest // SLICE
    dl = dest - SLICE * core
    w = dl >> 6
    dloc = dl & 63
    sec = (gidx >= LOHI).astype(np.int64)

    key = (core * 2 + sec) * NW + w
    order = np.lexsort((gidx, key))

    cnt = np.bincount(key[order], minlength=NC * 2 * NW).reshape(NC, 2, NW)
    wch = np.maximum(1, -(-cnt.max(axis=0) // 128))  # [2, NW]
    chunks_lo = wch[0]
    chunks_hi = wch[1]
    n_chunks = int(chunks_lo.sum() + chunks_hi.sum())
    E_PAD = 128 * n_chunks

    group_chunks = np.concatenate([chunks_lo, chunks_hi])
    group_off = np.zeros(2 * NW, dtype=np.int64)
    group_off[1:] = np.cumsum(group_chunks)[:-1] * 128

    per_core = []
    for k in range(NC):
        sel = order[core[order] == k]
        kgrp = sec[sel] * NW + w[sel]
        kcnt = np.bincount(kgrp, minlength=2 * NW)
        within = (
            np.concatenate([np.arange(c) for c in kcnt])
            if len(sel)
            else np.zeros(0, np.int64)
        )
        slot = group_off[kgrp] + within

        a_ef = np.zeros(E_PAD, dtype=np.float32)
        a_dloc = np.full(E_PAD, 200, dtype=np.float32)
        a_g16 = np.zeros(E_PAD, dtype=np.int16)
        a_l16 = np.zeros(E_PAD, dtype=np.int16)

        a_ef[slot] = ef[sel]
        a_dloc[slot] = dloc[sel]
        a_g16[slot] = (gidx[sel] - sec[sel] * LOHI).astype(np.int16)
        a_l16[slot] = dl[sel].astype(np.int16)

        per_core.append(
            dict(
                g16=_idx_layout(a_g16),
                l16=_idx_layout(a_l16),
                ef=a_ef.reshape(-1, 128).T.astype(BF).copy(),      # [128, nch]
                dloc=a_dloc.reshape(-1, 128).T.astype(BF).copy(),  # [128, nch]
            )
        )

    deg = np.bincount(dest, minlength=N).astype(np.float32)
    deg_local = np.zeros((NC, 1, SLICEP), np.float32)
    for k in range(NC):
        deg_local[k, 0, :SLICE] = deg[k * SLICE : (k + 1) * SLICE]

    meta = dict(
        chunks_lo=[int(x) for x in chunks_lo],
        chunks_hi=[int(x) for x in chunks_hi],
        n_chunks=n_chunks,
    )
    return meta, per_core, deg_local.astype(BF)


def _pad_features_blocks(feat):
    D = feat.shape[1]
    out = np.zeros((D, TROWS), np.float32)
    for k in range(NC):
        out[:, k * SLICEP : k * SLICEP + SLICE] = feat[k * SLICE : (k + 1) * SLICE].T
    return out.astype(BF)


def host_prep(inputs):
    ri = np.asarray(inputs["edge_indices"][0]).astype(np.int64)
    ci = np.asarray(inputs["edge_indices"][1]).astype(np.int64)
    ef = np.asarray(inputs["edge_features"]).reshape(-1).astype(np.float32)

    meta_v, pc_v, deg_r = _build_direction(ri, _g_of(ci), ef)
    meta_c, pc_c, deg_c = _build_direction(ci, _g_of(ri), ef)

    colF = np.asarray(inputs["col_features"], np.float32)
    rowF = np.asarray(inputs["row_features"], np.float32)

    colFT_own = np.zeros((NC, 19, OWNP), np.float32)
    rowFT_own = np.zeros((NC, 14, OWNP), np.float32)
    for k in range(NC):
        colFT_own[k, :, :SLICE] = colF[k * SLICE : (k + 1) * SLICE].T
        rowFT_own[k, :, :SLICE] = rowF[k * SLICE : (k + 1) * SLICE].T

    return dict(
        meta_v=meta_v, pc_v=pc_v, deg_r=deg_r,
        meta_c=meta_c, pc_c=pc_c, deg_c=deg_c,
        colFT=_pad_features_blocks(colF),
        colFT_own=colFT_own.astype(BF), rowFT_own=rowFT_own.astype(BF),
    )


def host_weights(inputs):
    f = lambda x: np.asarray(x, np.float32)
    b = lambda x: np.asarray(x, np.float32).astype(BF)
    v2c_W1 = f(inputs["v2c_W1"]); c2v_W1 = f(inputs["c2v_W1"])
    w = dict(
        col_W=b(inputs["col_W"]),
        col_b=f(inputs["col_b"]).reshape(64, 1),
        row_W=b(inputs["row_W"]),
        row_b=f(inputs["row_b"]).reshape(64, 1),
        W1a_v=v2c_W1[:64].astype(BF).copy(),
        W1c_v=v2c_W1[65:129].astype(BF).copy(),
        w1b_v=np.tile(v2c_W1[64:65], (128, 1)).astype(BF),
        b1row_v=f(inputs["v2c_b1"]).reshape(1, 64).astype(BF),
        W2aug_v=np.vstack([f(inputs["v2c_W2"]), f(inputs["v2c_b2"])[None, :]]).astype(BF),
        W2_v=b(inputs["v2c_W2"]),
        W1a_c=c2v_W1[:64].astype(BF).copy(),
        W1c_c=c2v_W1[65:129].astype(BF).copy(),
        w1b_c=np.tile(c2v_W1[64:65], (128, 1)).astype(BF),
        b1row_c=f(inputs["c2v_b1"]).reshape(1, 64).astype(BF),
        W2aug_c=np.vstack([f(inputs["c2v_W2"]), f(inputs["c2v_b2"])[None, :]]).astype(BF),
        W2_c=b(inputs["c2v_W2"]),
        out_W=b(inputs["out_W"]),
        out_b=f(inputs["out_b"]).reshape(1, 1),
        ones128=np.ones((1, 128), np.float32).astype(BF),
        iota64=np.tile(np.arange(64, dtype=np.float32)[None, :], (128, 1)).astype(BF),
    )
    return w


# ----------------------------------------------------------------------------
# kernel builder
# ----------------------------------------------------------------------------

def _chunk_plan(meta):
    """list over chunks of (sec, window, first_of_window, last_of_window)"""
    cw = []
    for sec, chunks in enumerate((meta["chunks_lo"], meta["chunks_hi"])):
        for w, c in enumerate(chunks):
            for j in range(c):
                cw.append((sec, w, j == 0, j == c - 1))
    return cw


def _call_plan(meta):
    """gather calls: (chunk0, nchunks) not crossing the lo/hi boundary"""
    nlo = sum(meta["chunks_lo"])
    nhi = sum(meta["chunks_hi"])
    calls = []
    for base, n in ((0, nlo), (nlo, nhi)):
        c = 0
        while c < n:
            cc = min(GC, n - c)
            calls.append((base + c, cc))
            c += cc
    return calls


STRIPES = [(i * 512, 512) for i in range(12)] + [(6144, 128)]


def build_kernel(meta_v, meta_c, repeat=1, skip_gathers=False, ablate=()):
    """repeat>1 builds a TIMING variant: both edge phases + epilogues wrapped
    in a hardware loop (collective hoisted out; scores not meaningful)."""
    nc = bacc.Bacc("TRN2", target_bir_lowering=False, debug=False, num_devices=NC,
                   dynamic_dma_scratch_size=32768, num_swdge_queues=2)

    def din(name, shape, dt=FP32):
        return nc.dram_tensor(name, shape, dt, kind="ExternalInput")

    colFT = din("colFT", [19, TROWS], BF16)
    colFT_own = din("colFT_own", [19, OWNP], BF16)
    rowFT_own = din("rowFT_own", [14, OWNP], BF16)
    col_W = din("col_W", [19, 64], BF16); col_b = din("col_b", [64, 1])
    row_W = din("row_W", [14, 64], BF16); row_b = din("row_b", [64, 1])
    W1a_v = din("W1a_v", [64, 64], BF16); W1c_v = din("W1c_v", [64, 64], BF16)
    w1b_v = din("w1b_v", [128, 64], BF16); b1row_v = din("b1row_v", [1, 64], BF16)
    W2aug_v = din("W2aug_v", [65, 64], BF16); W2_v = din("W2_v", [64, 64], BF16)
    W1a_c = din("W1a_c", [64, 64], BF16); W1c_c = din("W1c_c", [64, 64], BF16)
    w1b_c = din("w1b_c", [128, 64], BF16); b1row_c = din("b1row_c", [1, 64], BF16)
    W2aug_c = din("W2aug_c", [65, 64], BF16); W2_c = din("W2_c", [64, 64], BF16)
    out_W = din("out_W", [64, 1], BF16); out_b = din("out_b", [1, 1])
    ones128 = din("ones128", [1, 128], BF16)
    iota64 = din("iota64", [128, 64], BF16)

    ncv = meta_v["n_chunks"]; ncc = meta_c["n_chunks"]
    g16_v = din("g16_v", [128, ncv * 8], I16)
    l16_v = din("l16_v", [128, ncv * 8], I16)
    ef_v = din("ef_v", [128, ncv], BF16)
    dloc_v = din("dloc_v", [128, ncv], BF16)
    deg_r = din("deg_r", [1, SLICEP], BF16)
    g16_c = din("g16_c", [128, ncc * 8], I16)
    l16_c = din("l16_c", [128, ncc * 8], I16)
    ef_c = din("ef_c", [128, ncc], BF16)
    dloc_c = din("dloc_c", [128, ncc], BF16)
    deg_c = din("deg_c", [1, SLICEP], BF16)

    scores = nc.dram_tensor("scores", [SLICEP], FP32, kind="ExternalOutput")

    RELU = mybir.ActivationFunctionType.Relu
    COPY = mybir.ActivationFunctionType.Copy
    ADD = mybir.AluOpType.add
    MULT = mybir.AluOpType.mult
    EQ = mybir.AluOpType.is_equal

    with tile.TileContext(nc) as tc:
        with (
            tc.tile_pool(name="consts", bufs=1) as consts,
            tc.tile_pool(name="sb", bufs=2) as sb,
            tc.tile_pool(name="gath", bufs=2) as gath,
            tc.tile_pool(name="seg", bufs=1) as segp,
            tc.tile_pool(name="et", bufs=1) as etp,
            tc.tile_pool(name="ps_a", bufs=2, space="PSUM") as ps_a,
            tc.tile_pool(name="ps_b", bufs=2, space="PSUM") as ps_b,
            tc.tile_pool(name="ps_c", bufs=2, space="PSUM") as ps_c,
            tc.tile_pool(name="dram", bufs=1, space="DRAM") as dram,
        ):
            # ---- DRAM scratch. Gathered tables are bf16 padded to 512B
            # rows ([*, 256], only [:, :64] meaningful): 512B descriptors
            # run ~4x faster per descriptor than 256B on HW.
            colA_t = dram.tile([TROWS, 256], BF16)
            rowC_t = dram.tile([SLICEP, 256], BF16)
            colCp_t = dram.tile([SLICEP, 256], BF16)
            rowA_slice = dram.tile([SLICEP, 256], BF16)
            rowA_full = dram.tile([TROWS, 256], BF16)

            # ---- small consts
            def cload(dram_h, shape, dt=FP32):
                t = consts.tile(shape, dt, tag=f"c_{dram_h.name}")
                nc.sync.dma_start(t[:], dram_h[:])
                return t

            colW_s = cload(col_W, [19, 64], BF16); colb_s = cload(col_b, [64, 1])
            rowW_s = cload(row_W, [14, 64], BF16); rowb_s = cload(row_b, [64, 1])
            W1av_s = cload(W1a_v, [64, 64], BF16); W1cv_s = cload(W1c_v, [64, 64], BF16)
            w1bv_s = cload(w1b_v, [128, 64], BF16); b1v_s = cload(b1row_v, [1, 64], BF16)
            W2augv_s = cload(W2aug_v, [65, 64], BF16); W2v_s = cload(W2_v, [64, 64], BF16)
            W1ac_s = cload(W1a_c, [64, 64], BF16); W1cc_s = cload(W1c_c, [64, 64], BF16)
            w1bc_s = cload(w1b_c, [128, 64], BF16); b1c_s = cload(b1row_c, [1, 64], BF16)
            W2augc_s = cload(W2aug_c, [65, 64], BF16); W2c_s = cload(W2_c, [64, 64], BF16)
            outW_s = cload(out_W, [64, 1], BF16); outb_s = cload(out_b, [1, 1])
            ones_s = cload(ones128, [1, 128], BF16)
            iota_s = cload(iota64, [128, 64], BF16)

            # embedding tables for the epilogues, kept in SBUF
            colET = etp.tile([64, OWNP], BF16, tag="colET")
            rowET = etp.tile([64, OWNP], BF16, tag="rowET")

            # ---- phase 0: premultiplied node tables
            def emit_table(featT, D, Wemb_s, bemb_s, stripes, Wtab_s, table,
                           embT_out, brow_s=None):
                """embT = relu(Wemb.T @ featT + bemb); table = embT.T @ Wtab
                (+ brow broadcast over rows, via a rank-1 ones matmul)."""
                for (o, L) in stripes:
                    ft = sb.tile([D, 512], BF16, tag="ph0_ft")
                    nc.sync.dma_start(ft[:, :L], featT[:, o : o + L])
                    pe = ps_a.tile([64, 512], FP32, tag="ps_a")
                    nc.tensor.matmul(pe[:, :L], lhsT=Wemb_s[:], rhs=ft[:, :L],
                                     start=True, stop=True)
                    if embT_out is not None:
                        embT = embT_out[:, o : o + L]
                        nc.scalar.activation(embT, pe[:, :L], RELU, bias=bemb_s[:, :1])
                    else:
                        et = sb.tile([64, 512], BF16, tag="ph0_emb")
                        embT = et[:, :L]
                        nc.scalar.activation(embT, pe[:, :L], RELU, bias=bemb_s[:, :1])
                    nch = L // 128
                    pa = ps_b.tile([128, 8, 64], FP32, tag="ps_b")
                    for c in range(nch):
                        nc.tensor.matmul(
                            pa[:, c, :],
                            lhsT=embT[:, 128 * c : 128 * (c + 1)],
                            rhs=Wtab_s[:], start=True, stop=(brow_s is None),
                        )
                        if brow_s is not None:
                            nc.tensor.matmul(
                                pa[:, c, :], lhsT=ones_s[:], rhs=brow_s[:],
                                start=False, stop=True,
                            )
                    stage = sb.tile([128, 4, 64], BF16, tag="ph0_stage")
                    nc.scalar.activation(stage[:, :nch, :], pa[:, :nch, :], COPY)
                    nc.sync.dma_start(
                        table[o : o + L, 0:64].rearrange("(c p) h -> p c h", p=128),
                        stage[:, :nch, :],
                    )

            colA_stripes = [(i * 512, 512) for i in range(TROWS // 512)]    # 98
            emit_table(colFT, 19, colW_s, colb_s, colA_stripes, W1av_s, colA_t[:],
                       None, b1v_s)
            emit_table(colFT_own, 19, colW_s, colb_s, STRIPES, W1cc_s,
                       colCp_t[:], colET, b1c_s)
            emit_table(rowFT_own, 14, rowW_s, rowb_s, STRIPES, W1cv_s,
                       rowC_t[:], rowET)

            # ---- edge phase
            def edge_phase(meta, tab_lo, tab_hi, tab_loc, g16_d, l16_d, ef_d,
                           dloc_d, w1b_s, deg_d):
                cw = _chunk_plan(meta)
                calls = _call_plan(meta)
                nlo = sum(meta["chunks_lo"])
                nch = meta["n_chunks"]

                seg_lo = segp.tile([65, SLICEP], BF16, tag="seg_lo")
                seg_hi = segp.tile([64, SLICEP], BF16, tag="seg_hi")
                nc.sync.dma_start(seg_lo[64:65, :], deg_d[:])

                dlt_s = segp.tile([128, nch], BF16, tag="dlt")
                nc.sync.dma_start(dlt_s[:], dloc_d[:])
                eft_s = segp.tile([128, nch], BF16, tag="eft")
                nc.sync.dma_start(eft_s[:], ef_d[:])

                pw = None
                gt = lt = None
                idx_base = -1
                for ci_, (c0, ncall) in enumerate(calls):
                    is_lo = c0 < nlo
                    # batched idx loads (IDXC calls worth)
                    if ci_ % IDXC == 0:
                        nchb = 0
                        for (cc0, cnn) in calls[ci_ : ci_ + IDXC]:
                            nchb += cnn
                        nidxb = 128 * nchb
                        idx_base = c0
                        gt = sb.tile([128, GC * IDXC * 8], I16, tag="gidx")
                        nc.sync.dma_start(
                            gt[:, : nidxb // 16],
                            g16_d[:, c0 * 8 : c0 * 8 + nidxb // 16])
                        lt = sb.tile([128, GC * IDXC * 8], I16, tag="lidx")
                        nc.sync.dma_start(
                            lt[:, : nidxb // 16],
                            l16_d[:, c0 * 8 : c0 * 8 + nidxb // 16])

                    nidx = 128 * ncall
                    ib = (c0 - idx_base) * 8
                    no_gath = skip_gathers or "gathers" in ablate
                    gA = gath.tile([128, GC, 256], BF16, tag="gA")
                    gL = gath.tile([128, GC, 256], BF16, tag="gL")
                    if not no_gath:
                        nc.gpsimd.dma_gather(
                            gA[:, :ncall, :], (tab_lo if is_lo else tab_hi),
                            gt[:, ib : ib + nidx // 16],
                            num_idxs=nidx, num_idxs_reg=nidx, elem_size=256,
                        )
                        nc.gpsimd.dma_gather(
                            gL[:, :ncall, :], tab_loc,
                            lt[:, ib : ib + nidx // 16],
                            num_idxs=nidx, num_idxs_reg=nidx, elem_size=256,
                            queue_num=1,
                        )

                    for t0 in range(0, ncall, ST):
                        g = min(ST, ncall - t0)
                        cbase = c0 + t0

                        t1 = sb.tile([128, ST, 64], BF16, tag="t1")
                        nc.vector.tensor_tensor(
                            t1[:, :g, :],
                            w1b_s[:, None, :].to_broadcast([128, g, 64]),
                            eft_s[:, cbase : cbase + g, None].to_broadcast(
                                [128, g, 64]),
                            op=MULT,
                        )
                        if "tt" not in ablate and not no_gath:
                            nc.vector.tensor_tensor(
                                t1[:, :g, :], t1[:, :g, :],
                                gA[:, t0 : t0 + g, 0:64], op=ADD,
                            )
                            nc.vector.tensor_tensor(
                                t1[:, :g, :], t1[:, :g, :],
                                gL[:, t0 : t0 + g, 0:64], op=ADD,
                            )
                        if "s" not in ablate:
                            S = sb.tile([128, ST, 64], BF16, tag="S")
                            nc.vector.tensor_tensor(
                                S[:, :g, :],
                                dlt_s[:, cbase : cbase + g, None].to_broadcast(
                                    [128, g, 64]),
                                iota_s[:, None, :].to_broadcast([128, g, 64]),
                                op=EQ,
                            )

                        if "relu" not in ablate:
                            msg = sb.tile([128, ST, 64], BF16, tag="msg")
                            nc.scalar.activation(msg[:, :g, :], t1[:, :g, :], RELU)
                        else:
                            msg = t1

                        if "scatter" in ablate:
                            continue
                        for j in range(g):
                            c = cbase + j
                            sec, w, first, last = cw[c]
                            if w % 8 == 0 and first:
                                pw = ps_c.tile([64, 8, 64], FP32, tag="ps_c")
                            nc.tensor.matmul(
                                pw[:, w % 8, :],
                                lhsT=msg[:, j, :],
                                rhs=(S[:, j, :] if "s" not in ablate
                                     else iota_s[:, 0:64]),
                                start=first, stop=last,
                            )
                            if last and (w % 8 == 7 or w == NW - 1):
                                gw = 8 if w % 8 == 7 else (w % 8) + 1
                                wg = w - (w % 8)
                                acc = seg_lo if sec == 0 else seg_hi
                                if "segcopy" not in ablate:
                                    nc.scalar.activation(
                                        acc[0:64, WW * wg : WW * (wg + gw)],
                                        pw[:, :gw, :], COPY,
                                    )
                return seg_lo, seg_hi

            def epilogue(seg_lo, seg_hi, W2aug_s, W2_s_, ET, is_v2c):
                for (o, L) in STRIPES:
                    pn = ps_a.tile([64, 512], FP32, tag="ps_a")
                    nc.tensor.matmul(pn[:, :L], lhsT=W2aug_s[:],
                                     rhs=seg_lo[:, o : o + L], start=True, stop=False)
                    nc.tensor.matmul(pn[:, :L], lhsT=W2_s_[:],
                                     rhs=seg_hi[:, o : o + L], start=False, stop=True)
                    nT = sb.tile([64, 512], BF16, tag="nT")
                    nc.vector.tensor_tensor(nT[:, :L], pn[:, :L], ET[:, o : o + L],
                                            op=ADD)
                    if is_v2c:
                        nch = L // 128
                        pa = ps_b.tile([128, 8, 64], FP32, tag="ps_b")
                        for c in range(nch):
                            nc.tensor.matmul(
                                pa[:, c, :],
                                lhsT=nT[:, 128 * c : 128 * (c + 1)],
                                rhs=W1ac_s[:], start=True, stop=True,
                            )
                        stage = sb.tile([128, 4, 64], BF16, tag="rA_stage")
                        nc.scalar.activation(stage[:, :nch, :], pa[:, :nch, :], COPY)
                        nc.sync.dma_start(
                            rowA_slice[o : o + L, 0:64].rearrange(
                                "(c p) h -> p c h", p=128),
                            stage[:, :nch, :],
                        )
                    else:
                        psc = ps_c.tile([1, 512], FP32, tag="ps_c")
                        nc.tensor.matmul(psc[:, :L], lhsT=outW_s[:], rhs=nT[:, :L],
                                         start=True, stop=True)
                        sct = sb.tile([1, 512], FP32, tag="sc")
                        nc.vector.tensor_scalar(
                            out=sct[:, :L], in0=psc[:, :L],
                            scalar1=outb_s[:1, :1], scalar2=None, op0=ADD,
                        )
                        nc.sync.dma_start(scores[o : o + L], sct[:, :L])

            def v2c_block():
                seg_lo, seg_hi = edge_phase(
                    meta_v, colA_t[0:LOHI, :], colA_t[LOHI:TROWS, :], rowC_t[:],
                    g16_v, l16_v, ef_v, dloc_v, w1bv_s, deg_r,
                )
                epilogue(seg_lo, seg_hi, W2augv_s, W2v_s, rowET, True)

            def c2v_block():
                seg_lo, seg_hi = edge_phase(
                    meta_c, rowA_full[0:LOHI, :], rowA_full[LOHI:TROWS, :],
                    colCp_t[:], g16_c, l16_c, ef_c, dloc_c, w1bc_s, deg_c,
                )
                epilogue(seg_lo, seg_hi, W2augc_s, W2c_s, colET, False)

            def do_collective():
                nc.gpsimd.collective_compute(
                    "AllGather",
                    mybir.AluOpType.bypass,
                    replica_groups=[list(range(NC))],
                    ins=[rowA_slice.opt()],
                    outs=[rowA_full.opt()],
                )

            if repeat == 1:
                v2c_block()
                do_collective()
                c2v_block()
            else:
                v2c_block()
                do_collective()
                with tc.For_i(0, repeat, 1):
                    c2v_block()
                    v2c_block()

    nc.compile()
    return nc


# ----------------------------------------------------------------------------
# entry point
# ----------------------------------------------------------------------------

_CACHE = {}


def _get_kernel(meta_v, meta_c):
    key = (
        tuple(meta_v["chunks_lo"]), tuple(meta_v["chunks_hi"]),
        tuple(meta_c["chunks_lo"]), tuple(meta_c["chunks_hi"]),
    )
    if key not in _CACHE:
        _CACHE[key] = build_kernel(meta_v, meta_c)
    return _CACHE[key]


def make_in_maps(inputs, prep):
    w = host_weights(inputs)
    shared = dict(
        colFT=prep["colFT"],
        col_W=w["col_W"], col_b=w["col_b"], row_W=w["row_W"], row_b=w["row_b"],
        W1a_v=w["W1a_v"], W1c_v=w["W1c_v"], w1b_v=w["w1b_v"],
        b1row_v=w["b1row_v"],
        W2aug_v=w["W2aug_v"], W2_v=w["W2_v"],
        W1a_c=w["W1a_c"], W1c_c=w["W1c_c"], w1b_c=w["w1b_c"],
        b1row_c=w["b1row_c"],
        W2aug_c=w["W2aug_c"], W2_c=w["W2_c"],
        out_W=w["out_W"], out_b=w["out_b"], ones128=w["ones128"],
        iota64=w["iota64"],
    )
    in_maps = []
    for k in range(NC):
        pv, pc = prep["pc_v"][k], prep["pc_c"][k]
        m = dict(
            shared,
            colFT_own=prep["colFT_own"][k],
            rowFT_own=prep["rowFT_own"][k],
            g16_v=pv["g16"], l16_v=pv["l16"], ef_v=pv["ef"], dloc_v=pv["dloc"],
            deg_r=prep["deg_r"][k],
            g16_c=pc["g16"], l16_c=pc["l16"], ef_c=pc["ef"], dloc_c=pc["dloc"],
            deg_c=prep["deg_c"][k],
        )
        in_maps.append({kk: np.ascontiguousarray(vv) for kk, vv in m.items()})
    return in_maps


def kernel(**inputs):
    prep = host_prep(inputs)
    nc = _get_kernel(prep["meta_v"], prep["meta_c"])
    in_maps = make_in_maps(inputs, prep)
    res = run_bass_kernel_spmd(nc, in_maps, core_ids=list(range(NC)))
    out = np.zeros(N, np.float32)
    for k in range(NC):
        out[k * SLICE : (k + 1) * SLICE] = np.asarray(
            res.results[k]["scores"]).reshape(-1)[:SLICE]
    return out


# revision 19
# speedup vs baseline: 2.4135x; 1.0107x over previous
"""Trainium2 Bass kernel for the bipartite GCNN (8 NeuronCores, SPMD).

v2 design. Math identical to the reference:
  colE = relu(col_features @ col_W + col_b); rowE likewise.
  v2c: t1 = colA[ci] + rowC[ri] + ef*w1b + b1, colA = colE@W1a_v,
       rowC = rowE@W1c_v (premultiplied tables).
       msg = relu(t1); new_row = rowE + segsum(msg, ri) @ W2 + deg*b2
  c2v symmetric with rowA = new_rowE@W1a_c, colCp = colE@W1c_c.
  scores = new_col @ out_W + out_b

Sharding: destination-range; core k owns dest nodes [6250k, 6250(k+1)).
Edges sorted by (core, lo/hi-of-source, 64-wide dest window, source);
runs padded to a shared chunk plan so the SPMD program is identical.

Per 128-edge chunk (b1 baked into colA/colCp at phase0):
  DVE:  t1 = ef*w1b + gA + gL (3 TTs), S = (dloc == iota64) one-hot (64 wide)
  Act:  msg = relu(t1) -> bf16
  PE :  psum_seg[64, w%8*64:+64] += msg.T @ S   (the only per-chunk matmul)
  Act:  psum_seg -> seg (bf16), one copy per 8 windows
"""

import numpy as np
import ml_dtypes

import concourse.bass as bass
import concourse.mybir as mybir
import concourse.tile as tile
from concourse import bacc
from concourse.bass_utils import run_bass_kernel_spmd

NC = 8
N = 50000
SLICE = 6250
WW = 64                    # dest window width
NW = 98                    # windows per slice (98*64 = 6272)
SLICEP = NW * WW           # 6272
TROWS = NC * SLICEP        # 50176
OWNP = SLICEP              # own-block width for phase0 stripes
H = 64
LOHI = 32768
GC = 8                     # gather-call granularity in chunks (1024 descs)
ST = 8                     # compute supertile in chunks
IDXC = 8                   # gather calls per idx-load batch

FP32 = mybir.dt.float32
BF16 = mybir.dt.bfloat16
I16 = mybir.dt.int16
BF = ml_dtypes.bfloat16


# ----------------------------------------------------------------------------
# host-side preprocessing
# ----------------------------------------------------------------------------

def _g_of(n):
    return SLICEP * (n // SLICE) + n % SLICE


def _idx_layout(a):
    """slot array [E_PAD] -> dma_gather idx layout [128, E_PAD//16] int16"""
    A = a.reshape(-1, 16).T  # [16, E/16]
    return np.tile(A, (8, 1)).copy()


def _build_direction(dest, gidx, ef):
    core = dest // SLICE
    dl = dest - SLICE * core
    w = dl >> 6
    dloc = dl & 63
    sec = (gidx >= LOHI).astype(np.int64)

    key = (core * 2 + sec) * NW + w
    order = np.lexsort((gidx, key))

    cnt = np.bincount(key[order], minlength=NC * 2 * NW).reshape(NC, 2, NW)
    wch = np.maximum(1, -(-cnt.max(axis=0) // 128))  # [2, NW]
    chunks_lo = wch[0]
    chunks_hi = wch[1]
    n_chunks = int(chunks_lo.sum() + chunks_hi.sum())
    E_PAD = 128 * n_chunks

    group_chunks = np.concatenate([chunks_lo, chunks_hi])
    group_off = np.zeros(2 * NW, dtype=np.int64)
    group_off[1:] = np.cumsum(group_chunks)[:-1] * 128

    per_core = []
    for k in range(NC):
        sel = order[core[order] == k]
        kgrp = sec[sel] * NW + w[sel]
        kcnt = np.bincount(kgrp, minlength=2 * NW)
        within = (
            np.concatenate([np.arange(c) for c in kcnt])
            if len(sel)
            else np.zeros(0, np.int64)
        )
        slot = group_off[kgrp] + within

        a_ef = np.zeros(E_PAD, dtype=np.float32)
        a_dloc = np.full(E_PAD, 200, dtype=np.float32)
        a_g16 = np.zeros(E_PAD, dtype=np.int16)
        a_l16 = np.zeros(E_PAD, dtype=np.int16)

        a_ef[slot] = ef[sel]
        a_dloc[slot] = dloc[sel]
        a_g16[slot] = (gidx[sel] - sec[sel] * LOHI).astype(np.int16)
        a_l16[slot] = dl[sel].astype(np.int16)

        per_core.append(
            dict(
                g16=_idx_layout(a_g16),
                l16=_idx_layout(a_l16),
                ef=a_ef.reshape(-1, 128).T.astype(BF).copy(),      # [128, nch]
                dloc=a_dloc.reshape(-1, 128).T.astype(BF).copy(),  # [128, nch]
            )
        )

    deg = np.bincount(dest, minlength=N).astype(np.float32)
    deg_local = np.zeros((NC, 1, SLICEP), np.float32)
    for k in range(NC):
        deg_local[k, 0, :SLICE] = deg[k * SLICE : (k + 1) * SLICE]

    meta = dict(
        chunks_lo=[int(x) for x in chunks_lo],
        chunks_hi=[int(x) for x in chunks_hi],
        n_chunks=n_chunks,
    )
    return meta, per_core, deg_local.astype(BF)


def _pad_features_blocks(feat):
    D = feat.shape[1]
    out = np.zeros((D, TROWS), np.float32)
    for k in range(NC):
        out[:, k * SLICEP : k * SLICEP + SLICE] = feat[k * SLICE : (k + 1) * SLICE].T
    return out.astype(BF)


def host_prep(inputs):
    ri = np.asarray(inputs["edge_indices"][0]).astype(np.int64)
    ci = np.asarray(inputs["edge_indices"][1]).astype(np.int64)
    ef = np.asarray(inputs["edge_features"]).reshape(-1).astype(np.float32)

    meta_v, pc_v, deg_r = _build_direction(ri, _g_of(ci), ef)
    meta_c, pc_c, deg_c = _build_direction(ci, _g_of(ri), ef)

    colF = np.asarray(inputs["col_features"], np.float32)
    rowF = np.asarray(inputs["row_features"], np.float32)

    colFT_own = np.zeros((NC, 19, OWNP), np.float32)
    rowFT_own = np.zeros((NC, 14, OWNP), np.float32)
    for k in range(NC):
        colFT_own[k, :, :SLICE] = colF[k * SLICE : (k + 1) * SLICE].T
        rowFT_own[k, :, :SLICE] = rowF[k * SLICE : (k + 1) * SLICE].T

    return dict(
        meta_v=meta_v, pc_v=pc_v, deg_r=deg_r,
        meta_c=meta_c, pc_c=pc_c, deg_c=deg_c,
        colFT=_pad_features_blocks(colF),
        colFT_own=colFT_own.astype(BF), rowFT_own=rowFT_own.astype(BF),
    )


def host_weights(inputs):
    f = lambda x: np.asarray(x, np.float32)
    b = lambda x: np.asarray(x, np.float32).astype(BF)
    v2c_W1 = f(inputs["v2c_W1"]); c2v_W1 = f(inputs["c2v_W1"])
    w = dict(
        col_W=b(inputs["col_W"]),
        col_b=f(inputs["col_b"]).reshape(64, 1),
        row_W=b(inputs["row_W"]),
        row_b=f(inputs["row_b"]).reshape(64, 1),
        W1a_v=v2c_W1[:64].astype(BF).copy(),
        W1c_v=v2c_W1[65:129].astype(BF).copy(),
        w1b_v=np.tile(v2c_W1[64:65], (128, 1)).astype(BF),
        b1row_v=f(inputs["v2c_b1"]).reshape(1, 64).astype(BF),
        W2aug_v=np.vstack([f(inputs["v2c_W2"]), f(inputs["v2c_b2"])[None, :]]).astype(BF),
        W2_v=b(inputs["v2c_W2"]),
        W1a_c=c2v_W1[:64].astype(BF).copy(),
        W1c_c=c2v_W1[65:129].astype(BF).copy(),
        w1b_c=np.tile(c2v_W1[64:65], (128, 1)).astype(BF),
        b1row_c=f(inputs["c2v_b1"]).reshape(1, 64).astype(BF),
        W2aug_c=np.vstack([f(inputs["c2v_W2"]), f(inputs["c2v_b2"])[None, :]]).astype(BF),
        W2_c=b(inputs["c2v_W2"]),
        out_W=b(inputs["out_W"]),
        out_b=f(inputs["out_b"]).reshape(1, 1),
        ones128=np.ones((1, 128), np.float32).astype(BF),
        iota64=np.tile(np.arange(64, dtype=np.float32)[None, :], (128, 1)).astype(BF),
    )
    return w


# ----------------------------------------------------------------------------
# kernel builder
# ----------------------------------------------------------------------------

def _chunk_plan(meta):
    """list over chunks of (sec, window, first_of_window, last_of_window)"""
    cw = []
    for sec, chunks in enumerate((meta["chunks_lo"], meta["chunks_hi"])):
        for w, c in enumerate(chunks):
            for j in range(c):
                cw.append((sec, w, j == 0, j == c - 1))
    return cw


def _call_plan(meta):
    """gather calls: (chunk0, nchunks) not crossing the lo/hi boundary"""
    nlo = sum(meta["chunks_lo"])
    nhi = sum(meta["chunks_hi"])
    calls = []
    for base, n in ((0, nlo), (nlo, nhi)):
        c = 0
        while c < n:
            cc = min(GC, n - c)
            calls.append((base + c, cc))
            c += cc
    return calls


STRIPES = [(i * 512, 512) for i in range(12)] + [(6144, 128)]


def build_kernel(meta_v, meta_c, repeat=1, skip_gathers=False, ablate=()):
    """repeat>1 builds a TIMING variant: both edge phases + epilogues wrapped
    in a hardware loop (collective hoisted out; scores not meaningful)."""
    nc = bacc.Bacc("TRN2", target_bir_lowering=False, debug=False, num_devices=NC,
                   dynamic_dma_scratch_size=32768, num_swdge_queues=2)

    def din(name, shape, dt=FP32):
        return nc.dram_tensor(name, shape, dt, kind="ExternalInput")

    colFT = din("colFT", [19, TROWS], BF16)
    colFT_own = din("colFT_own", [19, OWNP], BF16)
    rowFT_own = din("rowFT_own", [14, OWNP], BF16)
    col_W = din("col_W", [19, 64], BF16); col_b = din("col_b", [64, 1])
    row_W = din("row_W", [14, 64], BF16); row_b = din("row_b", [64, 1])
    W1a_v = din("W1a_v", [64, 64], BF16); W1c_v = din("W1c_v", [64, 64], BF16)
    w1b_v = din("w1b_v", [128, 64], BF16); b1row_v = din("b1row_v", [1, 64], BF16)
    W2aug_v = din("W2aug_v", [65, 64], BF16); W2_v = din("W2_v", [64, 64], BF16)
    W1a_c = din("W1a_c", [64, 64], BF16); W1c_c = din("W1c_c", [64, 64], BF16)
    w1b_c = din("w1b_c", [128, 64], BF16); b1row_c = din("b1row_c", [1, 64], BF16)
    W2aug_c = din("W2aug_c", [65, 64], BF16); W2_c = din("W2_c", [64, 64], BF16)
    out_W = din("out_W", [64, 1], BF16); out_b = din("out_b", [1, 1])
    ones128 = din("ones128", [1, 128], BF16)
    iota64 = din("iota64", [128, 64], BF16)

    ncv = meta_v["n_chunks"]; ncc = meta_c["n_chunks"]
    g16_v = din("g16_v", [128, ncv * 8], I16)
    l16_v = din("l16_v", [128, ncv * 8], I16)
    ef_v = din("ef_v", [128, ncv], BF16)
    dloc_v = din("dloc_v", [128, ncv], BF16)
    deg_r = din("deg_r", [1, SLICEP], BF16)
    g16_c = din("g16_c", [128, ncc * 8], I16)
    l16_c = din("l16_c", [128, ncc * 8], I16)
    ef_c = din("ef_c", [128, ncc], BF16)
    dloc_c = din("dloc_c", [128, ncc], BF16)
    deg_c = din("deg_c", [1, SLICEP], BF16)

    scores = nc.dram_tensor("scores", [SLICEP], FP32, kind="ExternalOutput")

    RELU = mybir.ActivationFunctionType.Relu
    COPY = mybir.ActivationFunctionType.Copy
    ADD = mybir.AluOpType.add
    MULT = mybir.AluOpType.mult
    EQ = mybir.AluOpType.is_equal

    with tile.TileContext(nc) as tc:
        with (
            tc.tile_pool(name="consts", bufs=1) as consts,
            tc.tile_pool(name="sb", bufs=3) as sb,
            tc.tile_pool(name="gath", bufs=4) as gath,
            tc.tile_pool(name="seg", bufs=1) as segp,
            tc.tile_pool(name="et", bufs=1) as etp,
            tc.tile_pool(name="ps_a", bufs=2, space="PSUM") as ps_a,
            tc.tile_pool(name="ps_b", bufs=2, space="PSUM") as ps_b,
            tc.tile_pool(name="ps_c", bufs=2, space="PSUM") as ps_c,
            tc.tile_pool(name="dram", bufs=1, space="DRAM") as dram,
        ):
            # ---- DRAM scratch. Gathered tables are bf16 padded to 512B
            # rows ([*, 256], only [:, :64] meaningful): 512B descriptors
            # run ~4x faster per descriptor than 256B on HW.
            colA_t = dram.tile([TROWS, 256], BF16)
            rowC_t = dram.tile([SLICEP, 256], BF16)
            colCp_t = dram.tile([SLICEP, 256], BF16)
            rowA_slice = dram.tile([SLICEP, 256], BF16)
            rowA_full = dram.tile([TROWS, 256], BF16)

            # ---- small consts
            def cload(dram_h, shape, dt=FP32):
                t = consts.tile(shape, dt, tag=f"c_{dram_h.name}")
                nc.sync.dma_start(t[:], dram_h[:])
                return t

            colW_s = cload(col_W, [19, 64], BF16); colb_s = cload(col_b, [64, 1])
            rowW_s = cload(row_W, [14, 64], BF16); rowb_s = cload(row_b, [64, 1])
            W1av_s = cload(W1a_v, [64, 64], BF16); W1cv_s = cload(W1c_v, [64, 64], BF16)
            w1bv_s = cload(w1b_v, [128, 64], BF16); b1v_s = cload(b1row_v, [1, 64], BF16)
            W2augv_s = cload(W2aug_v, [65, 64], BF16); W2v_s = cload(W2_v, [64, 64], BF16)
            W1ac_s = cload(W1a_c, [64, 64], BF16); W1cc_s = cload(W1c_c, [64, 64], BF16)
            w1bc_s = cload(w1b_c, [128, 64], BF16); b1c_s = cload(b1row_c, [1, 64], BF16)
            W2augc_s = cload(W2aug_c, [65, 64], BF16); W2c_s = cload(W2_c, [64, 64], BF16)
            outW_s = cload(out_W, [64, 1], BF16); outb_s = cload(out_b, [1, 1])
            ones_s = cload(ones128, [1, 128], BF16)
            iota_s = cload(iota64, [128, 64], BF16)

            # embedding tables for the epilogues, kept in SBUF
            colET = etp.tile([64, OWNP], BF16, tag="colET")
            rowET = etp.tile([64, OWNP], BF16, tag="rowET")

            # ---- phase 0: premultiplied node tables
            def emit_table(featT, D, Wemb_s, bemb_s, stripes, Wtab_s, table,
                           embT_out, brow_s=None):
                """embT = relu(Wemb.T @ featT + bemb); table = embT.T @ Wtab
                (+ brow broadcast over rows, via a rank-1 ones matmul)."""
                for (o, L) in stripes:
                    ft = sb.tile([D, 512], BF16, tag="ph0_ft")
                    nc.sync.dma_start(ft[:, :L], featT[:, o : o + L])
                    pe = ps_a.tile([64, 512], FP32, tag="ps_a")
                    nc.tensor.matmul(pe[:, :L], lhsT=Wemb_s[:], rhs=ft[:, :L],
                                     start=True, stop=True)
                    if embT_out is not None:
                        embT = embT_out[:, o : o + L]
                        nc.scalar.activation(embT, pe[:, :L], RELU, bias=bemb_s[:, :1])
                    else:
                        et = sb.tile([64, 512], BF16, tag="ph0_emb")
                        embT = et[:, :L]
                        nc.scalar.activation(embT, pe[:, :L], RELU, bias=bemb_s[:, :1])
                    nch = L // 128
                    pa = ps_b.tile([128, 8, 64], FP32, tag="ps_b")
                    for c in range(nch):
                        nc.tensor.matmul(
                            pa[:, c, :],
                            lhsT=embT[:, 128 * c : 128 * (c + 1)],
                            rhs=Wtab_s[:], start=True, stop=(brow_s is None),
                        )
                        if brow_s is not None:
                            nc.tensor.matmul(
                                pa[:, c, :], lhsT=ones_s[:], rhs=brow_s[:],
                                start=False, stop=True,
                            )
                    stage = sb.tile([128, 4, 64], BF16, tag="ph0_stage")
                    nc.scalar.activation(stage[:, :nch, :], pa[:, :nch, :], COPY)
                    nc.sync.dma_start(
                        table[o : o + L, 0:64].rearrange("(c p) h -> p c h", p=128),
                        stage[:, :nch, :],
                    )

            colA_stripes = [(i * 512, 512) for i in range(TROWS // 512)]    # 98
            emit_table(colFT, 19, colW_s, colb_s, colA_stripes, W1av_s, colA_t[:],
                       None, b1v_s)
            emit_table(colFT_own, 19, colW_s, colb_s, STRIPES, W1cc_s,
                       colCp_t[:], colET, b1c_s)
            emit_table(rowFT_own, 14, rowW_s, rowb_s, STRIPES, W1cv_s,
                       rowC_t[:], rowET)

            # ---- edge phase
            def edge_phase(meta, tab_lo, tab_hi, tab_loc, g16_d, l16_d, ef_d,
                           dloc_d, w1b_s, deg_d):
                cw = _chunk_plan(meta)
                calls = _call_plan(meta)
                nlo = sum(meta["chunks_lo"])
                nch = meta["n_chunks"]

                seg_lo = segp.tile([65, SLICEP], BF16, tag="seg_lo")
                seg_hi = segp.tile([64, SLICEP], BF16, tag="seg_hi")
                nc.sync.dma_start(seg_lo[64:65, :], deg_d[:])

                dlt_s = segp.tile([128, nch], BF16, tag="dlt")
                nc.sync.dma_start(dlt_s[:], dloc_d[:])
                eft_s = segp.tile([128, nch], BF16, tag="eft")
                nc.sync.dma_start(eft_s[:], ef_d[:])

                pw = None
                gt = lt = None
                idx_base = -1
                for ci_, (c0, ncall) in enumerate(calls):
                    is_lo = c0 < nlo
                    # batched idx loads (IDXC calls worth)
                    if ci_ % IDXC == 0:
                        nchb = 0
                        for (cc0, cnn) in calls[ci_ : ci_ + IDXC]:
                            nchb += cnn
                        nidxb = 128 * nchb
                        idx_base = c0
                        gt = sb.tile([128, GC * IDXC * 8], I16, tag="gidx")
                        nc.sync.dma_start(
                            gt[:, : nidxb // 16],
                            g16_d[:, c0 * 8 : c0 * 8 + nidxb // 16])
                        lt = sb.tile([128, GC * IDXC * 8], I16, tag="lidx")
                        nc.sync.dma_start(
                            lt[:, : nidxb // 16],
                            l16_d[:, c0 * 8 : c0 * 8 + nidxb // 16])

                    nidx = 128 * ncall
                    ib = (c0 - idx_base) * 8
                    no_gath = skip_gathers or "gathers" in ablate
                    gA = gath.tile([128, GC, 256], BF16, tag="gA")
                    gL = gath.tile([128, GC, 256], BF16, tag="gL")
                    if not no_gath:
                        nc.gpsimd.dma_gather(
                            gA[:, :ncall, :], (tab_lo if is_lo else tab_hi),
                            gt[:, ib : ib + nidx // 16],
                            num_idxs=nidx, num_idxs_reg=nidx, elem_size=256,
                        )
                        nc.gpsimd.dma_gather(
                            gL[:, :ncall, :], tab_loc,
                            lt[:, ib : ib + nidx // 16],
                            num_idxs=nidx, num_idxs_reg=nidx, elem_size=256,
                            queue_num=1,
                        )

                    for t0 in range(0, ncall, ST):
                        g = min(ST, ncall - t0)
                        cbase = c0 + t0

                        t1 = sb.tile([128, ST, 64], BF16, tag="t1")
                        nc.vector.tensor_tensor(
                            t1[:, :g, :],
                            w1b_s[:, None, :].to_broadcast([128, g, 64]),
                            eft_s[:, cbase : cbase + g, None].to_broadcast(
                                [128, g, 64]),
                            op=MULT,
                        )
                        if "tt" not in ablate and not no_gath:
                            nc.vector.tensor_tensor(
                                t1[:, :g, :], t1[:, :g, :],
                                gA[:, t0 : t0 + g, 0:64], op=ADD,
                            )
                            nc.vector.tensor_tensor(
                                t1[:, :g, :], t1[:, :g, :],
                                gL[:, t0 : t0 + g, 0:64], op=ADD,
                            )
                        if "s" not in ablate:
                            S = sb.tile([128, ST, 64], BF16, tag="S")
                            nc.vector.tensor_tensor(
                                S[:, :g, :],
                                dlt_s[:, cbase : cbase + g, None].to_broadcast(
                                    [128, g, 64]),
                                iota_s[:, None, :].to_broadcast([128, g, 64]),
                                op=EQ,
                            )

                        if "relu" not in ablate:
                            msg = sb.tile([128, ST, 64], BF16, tag="msg")
                            nc.scalar.activation(msg[:, :g, :], t1[:, :g, :], RELU)
                        else:
                            msg = t1

                        if "scatter" in ablate:
                            continue
                        for j in range(g):
                            c = cbase + j
                            sec, w, first, last = cw[c]
                            if w % 8 == 0 and first:
                                pw = ps_c.tile([64, 8, 64], FP32, tag="ps_c")
                            nc.tensor.matmul(
                                pw[:, w % 8, :],
                                lhsT=msg[:, j, :],
                                rhs=(S[:, j, :] if "s" not in ablate
                                     else iota_s[:, 0:64]),
                                start=first, stop=last,
                            )
                            if last and (w % 8 == 7 or w == NW - 1):
                                gw = 8 if w % 8 == 7 else (w % 8) + 1
                                wg = w - (w % 8)
                                acc = seg_lo if sec == 0 else seg_hi
                                if "segcopy" not in ablate:
                                    nc.scalar.activation(
                                        acc[0:64, WW * wg : WW * (wg + gw)],
                                        pw[:, :gw, :], COPY,
                                    )
                return seg_lo, seg_hi

            def epilogue(seg_lo, seg_hi, W2aug_s, W2_s_, ET, is_v2c):
                for (o, L) in STRIPES:
                    pn = ps_a.tile([64, 512], FP32, tag="ps_a")
                    nc.tensor.matmul(pn[:, :L], lhsT=W2aug_s[:],
                                     rhs=seg_lo[:, o : o + L], start=True, stop=False)
                    nc.tensor.matmul(pn[:, :L], lhsT=W2_s_[:],
                                     rhs=seg_hi[:, o : o + L], start=False, stop=True)
                    nT = sb.tile([64, 512], BF16, tag="nT")
                    nc.vector.tensor_tensor(nT[:, :L], pn[:, :L], ET[:, o : o + L],
                                            op=ADD)
                    if is_v2c:
                        nch = L // 128
                        pa = ps_b.tile([128, 8, 64], FP32, tag="ps_b")
                        for c in range(nch):
                            nc.tensor.matmul(
                                pa[:, c, :],
                                lhsT=nT[:, 128 * c : 128 * (c + 1)],
                                rhs=W1ac_s[:], start=True, stop=True,
                            )
                        stage = sb.tile([128, 4, 64], BF16, tag="rA_stage")
                        nc.scalar.activation(stage[:, :nch, :], pa[:, :nch, :], COPY)
                        nc.sync.dma_start(
                            rowA_slice[o : o + L, 0:64].rearrange(
                                "(c p) h -> p c h", p=128),
                            stage[:, :nch, :],
                        )
                    else:
                        psc = ps_c.tile([1, 512], FP32, tag="ps_c")
                        nc.tensor.matmul(psc[:, :L], lhsT=outW_s[:], rhs=nT[:, :L],
                                         start=True, stop=True)
                        sct = sb.tile([1, 512], FP32, tag="sc")
                        nc.vector.tensor_scalar(
                            out=sct[:, :L], in0=psc[:, :L],
                            scalar1=outb_s[:1, :1], scalar2=None, op0=ADD,
                        )
                        nc.sync.dma_start(scores[o : o + L], sct[:, :L])

            def v2c_block():
                seg_lo, seg_hi = edge_phase(
                    meta_v, colA_t[0:LOHI, :], colA_t[LOHI:TROWS, :], rowC_t[:],
                    g16_v, l16_v, ef_v, dloc_v, w1bv_s, deg_r,
                )
                epilogue(seg_lo, seg_hi, W2augv_s, W2v_s, rowET, True)

            def c2v_block():
                seg_lo, seg_hi = edge_phase(
                    meta_c, rowA_full[0:LOHI, :], rowA_full[LOHI:TROWS, :],
                    colCp_t[:], g16_c, l16_c, ef_c, dloc_c, w1bc_s, deg_c,
                )
                epilogue(seg_lo, seg_hi, W2augc_s, W2c_s, colET, False)

            def do_collective():
                nc.gpsimd.collective_compute(
                    "AllGather",
                    mybir.AluOpType.bypass,
                    replica_groups=[list(range(NC))],
                    ins=[rowA_slice.opt()],
                    outs=[rowA_full.opt()],
                )

            if repeat == 1:
                v2c_block()
                do_collective()
                c2v_block()
            else:
                v2c_block()
                do_collective()
                with tc.For_i(0, repeat, 1):
                    c2v_block()
                    v2c_block()

    nc.compile()
    return nc


# ----------------------------------------------------------------------------
# entry point
# ----------------------------------------------------------------------------

_CACHE = {}


def _get_kernel(meta_v, meta_c):
    key = (
        tuple(meta_v["chunks_lo"]), tuple(meta_v["chunks_hi"]),
        tuple(meta_c["chunks_lo"]), tuple(meta_c["chunks_hi"]),
    )
    if key not in _CACHE:
        _CACHE[key] = build_kernel(meta_v, meta_c)
    return _CACHE[key]


def make_in_maps(inputs, prep):
    w = host_weights(inputs)
    shared = dict(
        colFT=prep["colFT"],
        col_W=w["col_W"], col_b=w["col_b"], row_W=w["row_W"], row_b=w["row_b"],
        W1a_v=w["W1a_v"], W1c_v=w["W1c_v"], w1b_v=w["w1b_v"],
        b1row_v=w["b1row_v"],
        W2aug_v=w["W2aug_v"], W2_v=w["W2_v"],
        W1a_c=w["W1a_c"], W1c_c=w["W1c_c"], w1b_c=w["w1b_c"],
        b1row_c=w["b1row_c"],
        W2aug_c=w["W2aug_c"], W2_c=w["W2_c"],
        out_W=w["out_W"], out_b=w["out_b"], ones128=w["ones128"],
        iota64=w["iota64"],
    )
    in_maps = []
    for k in range(NC):
        pv, pc = prep["pc_v"][k], prep["pc_c"][k]
        m = dict(
            shared,
            colFT_own=prep["colFT_own"][k],
            rowFT_own=prep["rowFT_own"][k],
            g16_v=pv["g16"], l16_v=pv["l16"], ef_v=pv["ef"], dloc_v=pv["dloc"],
            deg_r=prep["deg_r"][k],
            g16_c=pc["g16"], l16_c=pc["l16"], ef_c=pc["ef"], dloc_c=pc["dloc"],
            deg_c=prep["deg_c"][k],
        )
        in_maps.append({kk: np.ascontiguousarray(vv) for kk, vv in m.items()})
    return in_maps


def kernel(**inputs):
    prep = host_prep(inputs)
    nc = _get_kernel(prep["meta_v"], prep["meta_c"])
    in_maps = make_in_maps(inputs, prep)
    res = run_bass_kernel_spmd(nc, in_maps, core_ids=list(range(NC)))
    out = np.zeros(N, np.float32)
    for k in range(NC):
        out[k * SLICE : (k + 1) * SLICE] = np.asarray(
            res.results[k]["scores"]).reshape(-1)[:SLICE]
    return out
